# revision 49
# baseline (speedup 1.0000x reference)
"""Multi-head attention (B=8, N=1024, C=768, H=12, D=64) on 8 TRN2 NeuronCores.

Sharding: pure data parallel — one batch element per core, no collectives.

Per-core dataflow (all matmuls laid out so no on-device transposes are needed):
  A1: qT/kT feature-major:   qkvT[n,i] = sum_c wqkvT[c,n] * xT[c,i]
      (w stationary, xT moving).  q rows get *SCALE and +q_bias*SCALE on evict.
  A2: v token-major:         v[i,f] = sum_c xT[c,i] * wqkvT[c,f]
      (xT stationary, w moving).  Evicted bf16 into v_aug layout
      [tok, h*65+d] with a ones column per head at d=64.
  B:  per head pair g: S.T[j,i] = kT_h[d,j].T @ qT_h[d,i]  (K=64, heads 2g /
      2g+1 in partition rows 0:64 / 64:128 -> concurrent row-group matmuls).
      exp on the PSUM evict (no max subtraction; scores ~N(0,1)).
      O~.T[d,i] (+ softmax denominator row via the ones column) =
      v_aug[j,{d,1}].T @ expST[j,i], accumulated over j.
      Normalize: r = 1/s via reciprocal_approx_fast read straight from the
      PSUM denominator row, DRAM-bounce broadcast (f32), O~T *= r on gpsimd.
  C:  y[i,c] = sum_n OT[n,i].T @ wprojT[n,c]; evicted bf16 on the scalar
      engine; b_proj and the v_bias contribution (v_bias@w_proj.T, exact
      since softmax weights sum to 1) are added on the host.
"""

import sys

for p in ("/opt/trn_rl_repo", "/root/.axon_site/_ro/trn_rl_repo"):
    if p not in sys.path:
        sys.path.insert(0, p)

import numpy as np

import concourse.bacc as bacc
import concourse.mybir as mybir
import concourse.tile as tile

F32 = mybir.dt.float32
BF16 = mybir.dt.bfloat16

P = 128
NTOK = 1024
C = 768
H = 12
D = 64
SCALE = D ** -0.5
KT = C // P            # 6 contraction tiles over C
NT_Q = C // P          # 6 q feature tiles
JT = NTOK // P         # 8 key tiles
IT = NTOK // P         # 8 token tiles
VAW = H * (D + 1)      # 780: v_aug width


def build_attention(debug: bool = False):
    nc = bacc.Bacc("TRN2", target_bir_lowering=False, debug=False, num_devices=8)

    xT = nc.dram_tensor("xT", [C, NTOK], BF16, kind="ExternalInput")
    wqkvT = nc.dram_tensor("wqkvT", [C, 3 * C], BF16, kind="ExternalInput")
    wprojT = nc.dram_tensor("wprojT", [C, C], BF16, kind="ExternalInput")
    qkb = nc.dram_tensor("qkb", [P, NT_Q], F32, kind="ExternalInput")  # q_bias*SCALE, [p, nt]
    out = nc.dram_tensor("out", [C, NTOK], BF16, kind="ExternalOutput")  # y.T; host transposes

    dbg = {}
    if debug:
        dbg["ot"] = nc.dram_tensor("dbg_ot", [C, NTOK], BF16, kind="ExternalOutput")
        dbg["rb"] = nc.dram_tensor("dbg_rb", [4 * (H // 2), 512], F32, kind="ExternalOutput")

    with tile.TileContext(nc) as tc:
        with tc.tile_pool(name="persist", bufs=1) as persist, \
             tc.tile_pool(name="dramscratch", bufs=1, space="DRAM") as dramp:
            rbounce = dramp.tile([4 * (H // 2), 512], F32, tag="rbounce", name="rbounce")
            # persistent SBUF buffers
            qk_sb = [persist.tile([P, NTOK], BF16, tag=f"qk{t}", name=f"qk{t}") for t in range(12)]
            vaug_sb = [persist.tile([P, VAW], BF16, tag=f"vaug{t}", name=f"vaug{t}") for t in range(IT)]
            ot_sb = [persist.tile([P, NTOK], BF16, tag=f"ot{g}", name=f"ot{g}") for g in range(H // 2)]
            wp_sb = [persist.tile([P, C], BF16, tag=f"wp{t}", name=f"wp{t}") for t in range(KT)]
            qkb_sb = persist.tile([P, NT_Q], F32, tag="qkb")

            # ---------------- Phases A+B fused ----------------
            # A1 emits qT/kT in single-bank [128,512] PSUM groups.  Pair 0's
            # q/k tiles and all of v (A2) are a prologue; the remaining ten A1
            # groups stream through a generator that phase B pulls from (2
            # matmuls per step), filling the PE slack under the exp stream.
            # PSUM budget: psA 2 banks + pss 4 banks + pso 2 banks = 8.
            with tc.tile_pool(name="phA", bufs=1) as pa, \
                 tc.tile_pool(name="psA", bufs=2, space="PSUM") as psA, \
                 tc.tile_pool(name="phB", bufs=24) as pb, \
                 tc.tile_pool(name="phBs", bufs=2) as pbs, \
                 tc.tile_pool(name="psBs", bufs=2, space="PSUM") as psBs, \
                 tc.tile_pool(name="psBo", bufs=1, space="PSUM") as psBo:
                xT_sb = [pa.tile([P, NTOK], BF16, tag=f"xT{t}", name=f"xTsb{t}") for t in range(KT)]
                wq_sb = [pa.tile([P, 3 * C], BF16, tag=f"wq{t}", name=f"wqsb{t}") for t in range(KT)]

                # PE warm-up: ~40 dep-free tiny matmuls ramp the tensor engine
                # out of its low power state while the input DMAs stream in
                # (DVFS needs ~3us of sustained activity to reach full clock).
                warm = pa.tile([P, P], BF16, tag="warm", name="warm")
                wps = psA.tile([P, 512], F32, tag="psA1", name="warmps")
                nc.vector.memset(warm[:, :], 0.0)
                # 52 bridges the full window until the first A1 chain's DMAs
                # land (~10.5us) -- a gap here lets the DVFS ramp decay and
                # the prologue restarts at the slow clock.
                for _ in range(52):
                    nc.tensor.matmul(wps[:, 0:64], warm[:, :], warm[:, 0:64],
                                     start=True, stop=True)

                # Startup DMA priority order across three queues (host packs
                # wqkvT so each transfer is one contiguous chunk):
                #   cols 0:512    = [q-nt0 | k-nt0 | q-nt1 | k-nt1]  (pairs 0-1)
                #   cols 512:2304 = [q-nt2..5 | k-nt2..5 | v]
                # Critical path = the nt6/nt0 A1 chain: per kt, interleave the
                # q0|k0 slice with the first xT half so the kt-chain starts
                # pipelining behind the DMA stream at ~8us.
                #  sync (SP) even kt / scalar (Act) odd kt; gpsimd: qkb, wproj
                nc.gpsimd.dma_start(out=qkb_sb[:, :], in_=qkb[:, :])
                for t in range(KT):
                    q = nc.sync if t % 2 == 0 else nc.scalar
                    q.dma_start(out=wq_sb[t][:, 0:256], in_=wqkvT[t * P:(t + 1) * P, 0:256])
                    q.dma_start(out=xT_sb[t][:, 0:512], in_=xT[t * P:(t + 1) * P, 0:512])
                for t in range(KT):
                    q = nc.sync if t % 2 == 0 else nc.scalar
                    q.dma_start(out=xT_sb[t][:, 512:1024], in_=xT[t * P:(t + 1) * P, 512:1024])
                for t in range(KT):
                    q = nc.sync if t % 2 == 0 else nc.scalar
                    q.dma_start(out=wq_sb[t][:, 256:512], in_=wqkvT[t * P:(t + 1) * P, 256:512])
                for t in range(KT):
                    q = nc.sync if t % 2 == 0 else nc.scalar
                    q.dma_start(out=wq_sb[t][:, 512:2304], in_=wqkvT[t * P:(t + 1) * P, 512:2304])

                # packed wq column offsets: q/k feature-tile nt (0..5 = q, 6..11 = k)
                def wq_col(nt):
                    if nt == 0:
                        return 0
                    if nt == 6:
                        return P
                    if nt == 1:
                        return 2 * P
                    if nt == 7:
                        return 3 * P
                    if nt < 6:
                        return 512 + (nt - 2) * P
                    return 1024 + (nt - 8) * P

                def a1_groups_gen(nts):
                    for nt in nts:
                        for ib in range(2):
                            ps = psA.tile([P, 512], F32, tag="psA1", name="psa1")
                            for kt in range(KT):
                                nc.tensor.matmul(
                                    ps[:, :],
                                    wq_sb[kt][:, wq_col(nt):wq_col(nt) + P],
                                    xT_sb[kt][:, ib * 512:(ib + 1) * 512],
                                    start=(kt == 0), stop=(kt == KT - 1))
                                yield
                            if nt < NT_Q:
                                nc.vector.tensor_scalar(
                                    out=qk_sb[nt][:, ib * 512:(ib + 1) * 512], in0=ps[:, :],
                                    scalar1=float(SCALE), scalar2=qkb_sb[:, nt:nt + 1],
                                    op0=mybir.AluOpType.mult, op1=mybir.AluOpType.add)
                            else:
                                nc.vector.tensor_copy(
                                    qk_sb[nt][:, ib * 512:(ib + 1) * 512], ps[:, :])
                            yield

                # prologue: only the q/k tiles pair 0's S needs (plus nt7 so
                # the PE stays fed); everything else streams through phase B.
                for _ in a1_groups_gen((6, 0, 7)):
                    pass

                def a2_gen():
                    for tt in range(IT):
                        psv = [psA.tile([P, 384], F32, tag="psA1", name=f"psv{v}") for v in range(2)]
                        for kt in range(KT):
                            for vbk in range(2):
                                nc.tensor.matmul(
                                    psv[vbk][:, :],
                                    xT_sb[kt][:, tt * P:(tt + 1) * P],
                                    wq_sb[kt][:, 2 * C + vbk * 384: 2 * C + (vbk + 1) * 384],
                                    start=(kt == 0), stop=(kt == KT - 1))
                                yield
                        for vbk in range(2):
                            dst = vaug_sb[tt][:, :].rearrange("p (h e) -> p h e", e=D + 1)
                            nc.vector.tensor_copy(
                                dst[:, 6 * vbk:6 * (vbk + 1), 0:D],
                                psv[vbk][:, :].rearrange("p (h d) -> p h d", d=D))
                            yield
                        ones_cols = vaug_sb[tt][:, :].rearrange("p (h e) -> p h e", e=D + 1)[:, :, D:D + 1]
                        nc.vector.memset(ones_cols, 1.0)
                        yield

                for t in range(KT):
                    nc.gpsimd.dma_start(out=wp_sb[t][:, :], in_=wprojT[t * P:(t + 1) * P, :])

                import itertools
                awork = itertools.chain(
                    a2_gen(), a1_groups_gen((1, 2, 8, 3, 9, 4, 10, 5, 11)))

                def emit_o_mms(gp, pso, exp_, ib, jt):
                    for e in range(2):
                        nc.tensor.matmul(
                            pso[e][:, :],
                            vaug_sb[jt][:, (2 * gp + e) * (D + 1):(2 * gp + e + 1) * (D + 1)],
                            exp_[8 * ib + jt][:, e * 512:(e + 1) * 512],
                            start=(jt == 0), stop=(jt == JT - 1))

                def evict_o_half(gp, pso, ib, rp):
                    # O rows -> ot_sb (vector cast); denominator row: copy to
                    # partition 0 (custom-DVE ops drop the input AP's base
                    # partition), fast reciprocal, then DRAM-bounce broadcast
                    # into rp (f32).
                    for e in range(2):
                        nc.vector.tensor_copy(
                            ot_sb[gp][e * D:(e + 1) * D, ib * 512:(ib + 1) * 512],
                            pso[e][0:D, :])
                        raw = pbs.tile([1, 512], F32, tag=f"raw{e}", name=f"raw{e}", bufs=4)
                        if gp == H // 2 - 1 and ib == 1:
                            # final half: the scalar engine is idle (all exps
                            # done) -- extract the den row there so it runs
                            # parallel to the CAST above, shortening the
                            # normalize chain phase C's last tiles wait on.
                            nc.scalar.copy(out=raw[:, :], in_=pso[e][D:D + 1, :])
                        else:
                            nc.vector.tensor_copy(raw[:, :], pso[e][D:D + 1, :])
                        row = pbs.tile([1, 512], F32, tag=f"row{e}", name=f"row{e}", bufs=4)
                        nc.vector.reciprocal_approx_fast(out=row[:, :], in_=raw[:, :])
                        r = 4 * gp + 2 * ib + e
                        nc.sync.dma_start(out=rbounce[r:r + 1, :], in_=row[:, :])
                        nc.sync.dma_start(
                            out=rp[e * D:(e + 1) * D, ib * 512:(ib + 1) * 512],
                            in_=rbounce[r:r + 1, :].partition_broadcast(D))

                # Pair g's S/exp stream; pair g-1's O matmuls trail inside it
                # (sequential over ib so the O accumulators take 2 PSUM banks).
                prev = None  # (g, ex, rp) for cross-pair pairs 0..3
                A1_PULLS = {0: 8, 1: 4, 2: 2, 3: 2, 4: 1}
                SELF_PAIRS = (4, 5)  # O runs same-pair; normalized per-ib in-pair
                pending_tt = []      # deferred normalize multiplies (consumer is phase C)
                for g in range(H // 2):
                    self_o = g in SELF_PAIRS
                    ex = []
                    rp_self = None
                    if self_o:
                        rp_self = pbs.tile([P, NTOK], F32, tag="rpair", name=f"rpS{g}", bufs=3)
                    for dst, rp_ in pending_tt:
                        nc.gpsimd.tensor_mul(dst, dst, rp_)
                    pending_tt = []
                    for ib in range(2):
                        pso_prev = None
                        if prev is not None:
                            pso_prev = [psBo.tile([D + 1, 512], F32, tag=f"pso{e}", name=f"pso{e}")
                                        for e in range(2)]
                        pso_self = None
                        if self_o:
                            if g == 4:
                                pso_self = [psA.tile([D + 1, 512], F32, tag="psA1", name=f"psoL{e}")
                                            for e in range(2)]
                            else:
                                pso_self = [psBo.tile([D + 1, 512], F32, tag=f"pso{e}", name=f"psoM{e}")
                                            for e in range(2)]
                        for jt in range(JT):
                            for _ in range(A1_PULLS.get(g, 0)):
                                next(awork, None)
                            pss = psBs.tile([P, NTOK], F32, tag="pss")
                            for e in range(2):
                                nc.tensor.matmul(
                                    pss[:, e * 512:(e + 1) * 512],
                                    qk_sb[NT_Q + g][e * D:(e + 1) * D, jt * P:(jt + 1) * P],
                                    qk_sb[g][e * D:(e + 1) * D, ib * 512:(ib + 1) * 512],
                                    start=True, stop=True, tile_position=(e * D, 0))
                            et = pb.tile([P, NTOK], BF16, tag="expst")
                            nc.scalar.activation(et[:, :], pss[:, :], mybir.ActivationFunctionType.Exp)
                            ex.append(et)
                            if pso_prev is not None and jt > 0:
                                emit_o_mms(prev[0], pso_prev, prev[1], ib, jt - 1)
                            if pso_self is not None and jt > 0:
                                emit_o_mms(g, pso_self, ex, ib, jt - 1)
                        if pso_prev is not None:
                            emit_o_mms(prev[0], pso_prev, prev[1], ib, JT - 1)
                            evict_o_half(prev[0], pso_prev, ib, prev[2])
                        if pso_self is not None:
                            emit_o_mms(g, pso_self, ex, ib, JT - 1)
                            evict_o_half(g, pso_self, ib, rp_self)
                            tt = (ot_sb[g][:, ib * 512:(ib + 1) * 512],
                                  rp_self[:, ib * 512:(ib + 1) * 512])
                            if g == H // 2 - 1 and ib == 1:
                                nc.gpsimd.tensor_mul(tt[0], tt[0], tt[1])
                            else:
                                pending_tt.append(tt)
                    if prev is not None:
                        pending_tt.append((ot_sb[prev[0]][:, :], prev[2][:, :]))
                    if self_o:
                        prev = None
                    else:
                        rp = pbs.tile([P, NTOK], F32, tag="rpair", name=f"rp{g}", bufs=3)
                        prev = (g, ex, rp)
                for dst, rp_ in pending_tt:
                    nc.gpsimd.tensor_mul(dst, dst, rp_)
                pending_tt = []
                for _ in awork:
                    pass

                # ---------------- Phase C: output projection ----------------
                # Computed transposed: y.T[c,i] = sum_n wprojT[n,c] * OT[n,i],
                # so every matmul is a full 512-col stream (72 uniform matmuls
                # instead of 96 mixed 512/256), and the ib=0 token half only
                # depends on the pairs' ib=0 normalizes -- it overlaps the
                # tail of pair 5.  Shares PSUM slots with tag "pss" (no pool
                # barrier).  Evictions on the scalar engine (idle after the
                # last exp); bf16 out; host transposes back.
                psy_tags = ((psBs, "pss"), (psBs, "pss"), (psBo, "pso0"),
                            (psBo, "pso1"), (psA, "psA1"), (psA, "psA1"))
                ci = 0
                yt_tiles = [pb.tile([P, NTOK], BF16, tag=f"yout{ct}", bufs=1,
                                    name=f"yt{ct}")
                            for ct in range(KT)]
                for ib in range(2):
                    for ct in range(KT):
                        cpool, ctag = psy_tags[ci % 6]
                        ci += 1
                        psy = cpool.tile([P, 512], F32, tag=ctag, name="psy")
                        for gk in range(KT):
                            nc.tensor.matmul(
                                psy[:, :],
                                wp_sb[gk][:, ct * P:(ct + 1) * P],
                                ot_sb[gk][:, ib * 512:(ib + 1) * 512],
                                start=(gk == 0), stop=(gk == KT - 1))
                        # ib1 evictions alternate scalar/vector (both idle by
                        # then) and each half DMAs out as soon as it lands, so
                        # the post-matmul tail is one 96KB transfer, not six.
                        ev = nc.scalar if (ib == 0 or ct % 2 == 0) else nc.vector
                        if ev is nc.scalar:
                            ev.copy(out=yt_tiles[ct][:, ib * 512:(ib + 1) * 512], in_=psy[:, :])
                        else:
                            ev.tensor_copy(yt_tiles[ct][:, ib * 512:(ib + 1) * 512], psy[:, :])
                        # ib0 halves split across sync/gpsimd (overlapped
                        # mid-stream); all ib1 halves go on the fast HWDGE
                        # sync queue (idle by then) so the final transfer
                        # isn't stuck behind gpsimd's software-DGE issue.
                        q = nc.sync if (ib == 1 or ct % 2 == 0) else nc.gpsimd
                        q.dma_start(
                            out=out[ct * P:(ct + 1) * P, ib * 512:(ib + 1) * 512],
                            in_=yt_tiles[ct][:, ib * 512:(ib + 1) * 512])

                if debug:
                    for g in range(H // 2):
                        nc.sync.dma_start(out=dbg["ot"][g * P:(g + 1) * P, :], in_=ot_sb[g][:, :])
                    nc.gpsimd.dma_start(out=dbg["rb"][:, :], in_=rbounce[:, :])

    nc.compile()
    return nc


_CACHED_NC = {}


def get_compiled(debug: bool = False):
    if debug not in _CACHED_NC:
        _CACHED_NC[debug] = build_attention(debug)
    return _CACHED_NC[debug]


def prep_inputs(x, w_qkv, q_bias, v_bias, w_proj, b_proj):
    x = np.asarray(x, np.float32)
    B = x.shape[0]
    import ml_dtypes
    bf16 = ml_dtypes.bfloat16
    xT = np.ascontiguousarray(x.transpose(0, 2, 1)).astype(bf16)
    wqkvT_n = np.asarray(w_qkv, np.float32).T  # [C, 3C]: q | k | v feature-major
    # Pack columns so startup DMA is two contiguous chunks per kt row-block:
    #   [q-nt0 | k-nt0 | q-nt1 | k-nt1 | q-nt2..5 | k-nt2..5 | v]
    cols = []
    for nt in (0, 6, 1, 7):
        base = (nt % 6) * P + (C if nt >= 6 else 0)
        cols.append(wqkvT_n[:, base:base + P])
    cols.append(wqkvT_n[:, 2 * P:C])          # q nt2..5
    cols.append(wqkvT_n[:, C + 2 * P:2 * C])  # k nt2..5
    cols.append(wqkvT_n[:, 2 * C:3 * C])      # v
    wqkvT = np.ascontiguousarray(np.concatenate(cols, axis=1)).astype(bf16)
    wprojT = np.ascontiguousarray(np.asarray(w_proj, np.float32).T).astype(bf16)
    qkb = np.ascontiguousarray((np.asarray(q_bias, np.float32) * SCALE).reshape(NT_Q, P).T)
    return [
        {"xT": xT[i], "wqkvT": wqkvT, "wprojT": wprojT, "qkb": qkb}
        for i in range(B)
    ]


def host_bias_const(v_bias, w_proj, b_proj):
    # Softmax weights sum to 1, so v_bias shifts every attention output by
    # exactly v_bias; its contribution to y is v_bias @ w_proj.T + b_proj.
    vb = np.asarray(v_bias, np.float32)
    wp = np.asarray(w_proj, np.float32)
    bp = np.asarray(b_proj, np.float32)
    return vb @ wp.T + bp


def kernel(**inputs):
    """Harness entrypoint: full inputs in, full output out (8-way data parallel)."""
    from concourse.bass_utils import run_bass_kernel_spmd

    nc = get_compiled(False)
    in_maps = prep_inputs(
        inputs["x"], inputs["w_qkv"], inputs["q_bias"], inputs["v_bias"],
        inputs["w_proj"], inputs["b_proj"])
    res = run_bass_kernel_spmd(nc, in_maps, core_ids=list(range(8)))
    const = host_bias_const(inputs["v_bias"], inputs["w_proj"], inputs["b_proj"])
    return np.stack(
        [res.results[i]["out"].astype(np.float32).T + const for i in range(len(in_maps))])
